# revision 8
# baseline (speedup 1.0000x reference)
"""Trainium2 Bass kernel for Mistral-style sliding-window GQA attention.

Problem (hardcoded): B=1, S=4096, E=2048, H=16 q-heads, G=4 kv-heads,
D=128, sliding window W=1024 (q_idx - kv_idx in [0, W], kv_idx >= 0).

Sharding: pure sequence-parallel over 8 NeuronCores, 512 queries per core.
Each core recomputes the K/V projection for its 1024-position halo (the
host hands it a zero-padded, pre-transposed input slice), so no
collectives are needed; the host concatenates the 8 output shards.

Per-core pipeline (all matmuls in float32r — 1 PE cycle/row, ~1.5e-4 rel
err — with fp32 PSUM accumulation):
  A1  K/V projection for all 1536 local positions + RoPE(K) + PE-transpose
      K into kT[d, s] layout; V kept natural [s, d].
  A2  Q projection for own 512 positions (+RoPE, transpose to qT[d, s]).
      The 1/sqrt(D) attention scale is folded into w_in's Q columns on
      the host.
  B   Per (head, 128-query tile): banded scores [128, 1152] via 3 PE
      matmuls into a strided PSUM view; band-edge triangle masks added in
      PSUM by DVE; one ACT Exp with free-axis accumulation gives probs +
      row sums; padding is handled with zero K/V columns (score 0 ->
      exp=1, V=0) plus a host-provided per-query count correction folded
      into the ACT Reciprocal bias; DVE normalizes probs; PE transposes
      probs into a zero-initialized pT[kv, q] buffer; PV accumulates
      o^T[d, q] per head over 12 kv tiles.
  C   Output projection: out[s, e] accumulated over the 16 heads with
      o^T tiles as stationary operands; natural-layout stores.
"""

import numpy as np

# ---------------- problem constants ----------------
B, S, E = 1, 4096, 2048
H, G, D = 16, 4, 128
W = 1024
NCORES = 8
S_SH = S // NCORES            # 512 queries per core
L = S_SH + W                  # 1536 local kv positions (incl. halo)
NT = S_SH // 128              # 4 query tiles
NJ = L // 128                 # 12 kv tiles
KV = G * D                    # 512
NEG = -1.0e9

_CACHE = {}


# ---------------- walrus multi-wait workaround ----------------
def _patch_tile():
    """This walrus build rejects instructions with >1 sync waits
    ("Too many sync wait commands").  Patch TileContext so that after
    scheduling, excess waits on compute-engine instructions are hoisted
    onto same-engine nops placed immediately before them (engine program
    order preserves the barrier), and excess waits on DMA instructions are
    converted into merge-nops on the producing engines that increment one
    global merge semaphore the DMA waits on.  Merge-nops sit at the DMA's
    stream position, where Tile's original schedule already guarantees
    every hoisted condition is satisfiable, so no deadlock is introduced.
    """
    import concourse.mybir as mybir
    from concourse import tile as _tile

    if getattr(_tile.TileContext, "_wait_split_patched", False):
        return

    eng_prefixes = [
        ("Activation", mybir.EngineType.Activation),
        ("DVE", mybir.EngineType.DVE),
        ("PE", mybir.EngineType.PE),
        ("Pool", mybir.EngineType.Pool),
        ("SP", mybir.EngineType.SP),
    ]

    def wait_engine(w):
        name = w.ant_name or ""
        for pfx, e in eng_prefixes:
            if name.startswith(pfx):
                return e
        return mybir.EngineType.SP

    def is_dma(inst):
        n = type(inst).__name__
        return "DMA" in n or "TensorLoad" in n or "TensorSave" in n

    counter = [0]

    def split_multi_waits(nc, limit=1):
        for f in nc.m.functions:
            for blk in f.blocks:
                insts = list(blk.instructions)
                needs = {
                    i.name
                    for i in insts
                    if i.sync_info is not None
                    and i.sync_info.on_wait
                    and len(i.sync_info.on_wait) > limit
                }
                if not needs:
                    continue
                new_order = []
                for inst in insts:
                    if inst.name in needs:
                        si = inst.sync_info
                        waits = list(si.on_wait)
                        if is_dma(inst):
                            sem = nc._waitmerge_sem
                            for w in waits:
                                e = wait_engine(w)
                                h = nc.engines[e].nop(
                                    hint="wait_merge", nofuse=True
                                )
                                h.then_inc(sem, 1)
                                nc._waitmerge_count += 1
                                nop = h.ins
                                upd = (
                                    list(nop.sync_info.on_update)
                                    if nop.sync_info
                                    else []
                                )
                                nop.sync_info = mybir.SyncInfo(
                                    on_wait=[w], on_update=upd
                                )
                                new_order.append(nop)
                            si.on_wait = [
                                mybir.SyncWait(
                                    sync_type="semaphore",
                                    id=sem.num,
                                    ant_name=sem.name,
                                    wait_mode="sem-ge-imm",
                                    wait_value=nc._waitmerge_count,
                                    wait_reg=None,
                                )
                            ]
                        else:
                            for w in waits[:-limit]:
                                counter[0] += 1
                                nop = mybir.InstNoOp(
                                    name=f"I-waitsplit-{counter[0]}",
                                    ins=[],
                                    outs=[],
                                )
                                nop.engine = inst.engine
                                nop.sync_info = mybir.SyncInfo(
                                    on_wait=[w], on_update=[]
                                )
                                nc.register_instruction(nop)
                                blk.add_instruction(nop)
                                new_order.append(nop)
                            si.on_wait = waits[-limit:]
                    new_order.append(inst)
                seen = set()
                rebuilt = []
                for i in new_order:
                    if i.name not in seen:
                        seen.add(i.name)
                        rebuilt.append(i)
                blk.instructions = rebuilt
        # drop stray duplicates of merge-nops appended to other blocks
        placed = set()
        for f in nc.m.functions:
            for blk in f.blocks:
                insts = list(blk.instructions)
                out = [i for i in insts if i.name not in placed]
                for i in out:
                    placed.add(i.name)
                if len(out) != len(insts):
                    blk.instructions = out

    orig_enter = _tile.TileContext.__enter__
    orig_exit = _tile.TileContext.__exit__

    def _enter(self):
        if not hasattr(self.nc, "_waitmerge_sem"):
            self.nc._waitmerge_sem = self.nc.alloc_semaphore("waitmerge")
            self.nc._waitmerge_count = 0
        return orig_enter(self)

    def _exit(self, exc_type, exc_value, tb):
        r = orig_exit(self, exc_type, exc_value, tb)
        if exc_type is None:
            split_multi_waits(self.nc)
        return r

    _tile.TileContext.__enter__ = _enter
    _tile.TileContext.__exit__ = _exit
    _tile.TileContext._wait_split_patched = True


# ---------------- device program ----------------
def _rep_free(ap, reps):
    """View a [128, 64] AP as [128, reps, 64] via a step-0 repeat dim."""
    import concourse.bass as bass

    dims = [list(d) for d in ap.ap]
    assert len(dims) == 2
    return bass.AP(ap.tensor, ap.offset, [dims[0], [0, reps], dims[1]])


def _build_program():
    import concourse.bass as bass
    import concourse.mybir as mybir
    from concourse.tile import TileContext
    from concourse.masks import make_identity

    _patch_tile()

    F32 = mybir.dt.float32
    F32R = mybir.dt.float32r
    AF = mybir.ActivationFunctionType
    OP = mybir.AluOpType

    nc = bass.Bass(
        "TRN2", target_bir_lowering=False, debug=False, num_devices=NCORES
    )
    xT = nc.dram_tensor("xT", [E, L], F32R, kind="ExternalInput").ap()
    w_in = nc.dram_tensor("w_in", [E, E + 2 * KV], F32R, kind="ExternalInput").ap()
    w_out = nc.dram_tensor("w_out", [E, E], F32R, kind="ExternalInput").ap()
    cos_k = nc.dram_tensor("cos_k", [L, 64], F32, kind="ExternalInput").ap()
    sin_k = nc.dram_tensor("sin_k", [L, 64], F32, kind="ExternalInput").ap()
    maskLR = nc.dram_tensor("maskLR", [128, 256], F32, kind="ExternalInput").ap()
    sumfix = nc.dram_tensor("sumfix", [S_SH, 1], F32, kind="ExternalInput").ap()
    out = nc.dram_tensor("out", [S_SH, E], F32, kind="ExternalOutput").ap()

    def rope_block(nc, pool, out_sb, in_ps, cos64, sin64, reps):
        """out = RoPE(in) for a [128, reps*128] block; pairs interleaved
        along free dim; cos/sin are [128, 64] APs repeated `reps`x."""
        m = reps * 64
        i3 = in_ps.rearrange("p (r mt) -> p r mt", r=reps)
        o3 = out_sb.rearrange("p (r mt) -> p r mt", r=reps)
        xe, xo = i3[:, :, 0::2], i3[:, :, 1::2]
        oe, oo = o3[:, :, 0::2], o3[:, :, 1::2]
        cr = _rep_free(cos64, reps)
        sr = _rep_free(sin64, reps)
        t1 = pool.tile([128, reps, 64], F32, name="ropeT1", tag="ropeT1")
        t2 = pool.tile([128, reps, 64], F32, name="ropeT2", tag="ropeT2")
        nc.vector.tensor_tensor(t1[:], xe, cr, OP.mult)
        nc.vector.tensor_tensor(t2[:], xo, sr, OP.mult)
        nc.vector.tensor_tensor(oe, t1[:], t2[:], OP.subtract)
        nc.vector.tensor_tensor(t1[:], xe, sr, OP.mult)
        nc.vector.tensor_tensor(t2[:], xo, cr, OP.mult)
        nc.vector.tensor_tensor(oo, t1[:], t2[:], OP.add)

    with TileContext(nc) as tc:
        with tc.tile_pool(name="misc", bufs=1) as misc, \
             tc.tile_pool(name="pool_kv", bufs=1) as pool_kv:
            kT = [
                pool_kv.tile([128, L], F32R, name=f"kT{g}", tag=f"kT{g}")
                for g in range(G)
            ]
            vv = [
                pool_kv.tile([128, KV], F32R, name=f"v{j}", tag=f"v{j}")
                for j in range(NJ)
            ]
            ident = misc.tile([128, 128], F32, tag="ident")
            make_identity(nc, ident[:])
            mlr = misc.tile([128, 256], F32, tag="mlr")
            nc.sync.dma_start(out=mlr[:], in_=maskLR[:])
            sfx = misc.tile([128, NT], F32, tag="sfx")
            nc.sync.dma_start(
                out=sfx[:].rearrange("p (t o) -> p t o", t=NT),
                in_=sumfix.rearrange("(t p) o -> p t o", p=128),
            )
            cosk = misc.tile([128, NJ * 64], F32, tag="cosk")
            sink = misc.tile([128, NJ * 64], F32, tag="sink")
            nc.sync.dma_start(
                out=cosk[:].rearrange("p (t m) -> p t m", t=NJ),
                in_=cos_k.rearrange("(t p) m -> p t m", p=128),
            )
            nc.sync.dma_start(
                out=sink[:].rearrange("p (t m) -> p t m", t=NJ),
                in_=sin_k.rearrange("(t p) m -> p t m", p=128),
            )

            # ---------------- Phase A1: K/V projection ----------------
            with tc.tile_pool(name="xin", bufs=3) as xpool, \
                 tc.tile_pool(name="wkv", bufs=1) as wkvp, \
                 tc.tile_pool(name="ropeA", bufs=4) as rtp, \
                 tc.tile_pool(name="psA", bufs=2, space="PSUM") as psA, \
                 tc.tile_pool(name="psTA", bufs=2, space="PSUM") as psTA:
                wkv = [
                    wkvp.tile(
                        [128, 2 * KV], F32R, name=f"wkv{et}", tag=f"wkv{et}"
                    )
                    for et in range(16)
                ]
                for et in range(16):
                    nc.sync.dma_start(
                        out=wkv[et][:],
                        in_=w_in[et * 128:(et + 1) * 128, E:E + 2 * KV],
                    )
                for st in range(NJ):
                    xt = xpool.tile([128, 16, 128], F32R, name="xh", tag="xh")
                    nc.sync.dma_start(
                        out=xt[:],
                        in_=xT[:, st * 128:(st + 1) * 128].rearrange(
                            "(a p) c -> p a c", p=128
                        ),
                    )
                    psk = psA.tile([128, KV], F32, tag="psk")
                    psv = psA.tile([128, KV], F32, tag="psv")
                    for et in range(16):
                        nc.tensor.matmul(
                            psk[:], xt[:, et, :], wkv[et][:, 0:KV],
                            start=(et == 0), stop=(et == 15),
                        )
                    for et in range(16):
                        nc.tensor.matmul(
                            psv[:], xt[:, et, :], wkv[et][:, KV:2 * KV],
                            start=(et == 0), stop=(et == 15),
                        )
                    nc.scalar.copy(out=vv[st][:], in_=psv[:])
                    kr = rtp.tile([128, KV], F32, name="kr", tag="kr")
                    rope_block(
                        nc, rtp, kr[:], psk[:],
                        cosk[:, st * 64:(st + 1) * 64],
                        sink[:, st * 64:(st + 1) * 64],
                        reps=G,
                    )
                    pst = psTA.tile([128, KV], F32, tag="pstA")
                    for g in range(G):
                        nc.tensor.transpose(
                            pst[:, g * 128:(g + 1) * 128],
                            kr[:, g * 128:(g + 1) * 128],
                            ident[:],
                        )
                    for g in range(G):
                        nc.scalar.copy(
                            out=kT[g][:, st * 128:(st + 1) * 128],
                            in_=pst[:, g * 128:(g + 1) * 128],
                        )

            with tc.tile_pool(name="pool_q", bufs=1) as pool_q, \
                 tc.tile_pool(name="pool_o", bufs=1) as pool_o:
                qT = [
                    pool_q.tile([128, S_SH], F32R, name=f"qT{h}", tag=f"qT{h}")
                    for h in range(H)
                ]
                oT = [
                    pool_o.tile([128, S_SH], F32R, name=f"oT{h}", tag=f"oT{h}")
                    for h in range(H)
                ]

                # ---------------- Phase A2: Q projection ----------------
                with tc.tile_pool(name="xown", bufs=1) as xop, \
                     tc.tile_pool(name="wq", bufs=6) as wqp, \
                     tc.tile_pool(name="ropeQ", bufs=4) as rqp, \
                     tc.tile_pool(name="psQ", bufs=1, space="PSUM") as psQ, \
                     tc.tile_pool(name="psTQ", bufs=2, space="PSUM") as psTQ:
                    x_own = [
                        xop.tile(
                            [128, 16, 128], F32R, name=f"xo{i}", tag=f"xo{i}"
                        )
                        for i in range(NT)
                    ]
                    for i in range(NT):
                        st = 8 + i
                        nc.sync.dma_start(
                            out=x_own[i][:],
                            in_=xT[:, st * 128:(st + 1) * 128].rearrange(
                                "(a p) c -> p a c", p=128
                            ),
                        )
                    for fb in range(4):
                        psq = [
                            psQ.tile(
                                [128, 512], F32, name=f"psq{i}", tag=f"psq{i}"
                            )
                            for i in range(NT)
                        ]
                        for et in range(16):
                            wq = wqp.tile([128, 512], F32R, name="wq", tag="wq")
                            nc.sync.dma_start(
                                out=wq[:],
                                in_=w_in[et * 128:(et + 1) * 128,
                                         fb * 512:(fb + 1) * 512],
                            )
                            for i in range(NT):
                                nc.tensor.matmul(
                                    psq[i][:], x_own[i][:, et, :], wq[:],
                                    start=(et == 0), stop=(et == 15),
                                )
                        for i in range(NT):
                            qr = rqp.tile([128, 512], F32, name="qr", tag="qr")
                            rope_block(
                                nc, rqp, qr[:], psq[i][:],
                                cosk[:, (8 + i) * 64:(9 + i) * 64],
                                sink[:, (8 + i) * 64:(9 + i) * 64],
                                reps=4,
                            )
                            pst = psTQ.tile([128, 512], F32, tag="pstQ")
                            for r in range(4):
                                nc.tensor.transpose(
                                    pst[:, r * 128:(r + 1) * 128],
                                    qr[:, r * 128:(r + 1) * 128],
                                    ident[:],
                                )
                            for r in range(4):
                                h = fb * 4 + r
                                nc.scalar.copy(
                                    out=qT[h][:, i * 128:(i + 1) * 128],
                                    in_=pst[:, r * 128:(r + 1) * 128],
                                )

                # ---------------- Phase B: attention ----------------
                with tc.tile_pool(name="pool_pt", bufs=1) as pool_pt, \
                     tc.tile_pool(name="probs", bufs=4) as prp, \
                     tc.tile_pool(name="small", bufs=8) as smp, \
                     tc.tile_pool(name="psS", bufs=2, space="PSUM") as psS, \
                     tc.tile_pool(name="psPV", bufs=1, space="PSUM") as psPV, \
                     tc.tile_pool(name="psTB", bufs=1, space="PSUM") as psTB:
                    pT = [
                        pool_pt.tile(
                            [128, S_SH], F32R, name=f"pT{j}", tag=f"pT{j}"
                        )
                        for j in range(NJ)
                    ]
                    for j in list(range(3)) + list(range(9, 12)):
                        nc.gpsimd.memset(pT[j][:].bitcast(F32), 0.0)
                    for h in range(H):
                        g = h // 4
                        for t in range(NT):
                            pss = psS.tile([128, 3, 512], F32, tag="pss")
                            for bb in range(3):
                                off = t * 128 + bb * 384
                                nc.tensor.matmul(
                                    pss[:, bb, 0:384],
                                    qT[h][:, t * 128:(t + 1) * 128],
                                    kT[g][:, off:off + 384],
                                    start=True, stop=True,
                                )
                            nc.vector.tensor_tensor(
                                pss[:, 0, 0:128], pss[:, 0, 0:128],
                                mlr[:, 0:128], OP.add,
                            )
                            nc.vector.tensor_tensor(
                                pss[:, 2, 256:384], pss[:, 2, 256:384],
                                mlr[:, 128:256], OP.add,
                            )
                            probs = prp.tile(
                                [128, 1152], F32, name="probs", tag="probs"
                            )
                            sums = smp.tile([128, 1], F32, name="sums", tag="sums")
                            nc.scalar.activation(
                                probs[:].rearrange("p (a bb) -> p a bb", a=3),
                                pss[:, :, 0:384],
                                AF.Exp, scale=1.0, accum_out=sums[:],
                            )
                            recip = smp.tile(
                                [128, 1], F32, name="recip", tag="recip"
                            )
                            nc.vector.tensor_tensor(
                                recip[:], sums[:], sfx[:, t:t + 1], OP.add
                            )
                            nc.vector.reciprocal(recip[:], recip[:])
                            nc.vector.tensor_scalar_mul(
                                probs[:], probs[:], recip[:]
                            )
                            for ch in range(3):
                                nblk = 4 if ch < 2 else 1
                                pstb = psTB.tile([128, 512], F32, tag="pstB")
                                for q in range(nblk):
                                    jj = ch * 4 + q
                                    nc.tensor.transpose(
                                        pstb[:, q * 128:(q + 1) * 128],
                                        probs[:, jj * 128:(jj + 1) * 128],
                                        ident[:],
                                    )
                                for q in range(nblk):
                                    jj = ch * 4 + q
                                    nc.vector.tensor_copy(
                                        out=pT[t + jj][:, t * 128:(t + 1) * 128],
                                        in_=pstb[:, q * 128:(q + 1) * 128],
                                    )
                        pspv = psPV.tile([128, 512], F32, tag="pspv")
                        for jt in range(NJ):
                            nc.tensor.matmul(
                                pspv[:],
                                vv[jt][:, g * 128:(g + 1) * 128],
                                pT[jt][:],
                                start=(jt == 0), stop=(jt == NJ - 1),
                            )
                        nc.scalar.copy(out=oT[h][:], in_=pspv[:])

                # ---------------- Phase C: output projection ----------------
                with tc.tile_pool(name="wo", bufs=18) as wop, \
                     tc.tile_pool(name="osb", bufs=3) as osbp, \
                     tc.tile_pool(name="psC", bufs=2, space="PSUM") as psC:
                    for eseg in range(4):
                        wo = [
                            wop.tile([128, 512], F32R, name="wo", tag="wo")
                            for _ in range(16)
                        ]
                        for fh in range(16):
                            nc.sync.dma_start(
                                out=wo[fh][:],
                                in_=w_out[fh * 128:(fh + 1) * 128,
                                          eseg * 512:(eseg + 1) * 512],
                            )
                        for st in range(NT):
                            pso = psC.tile([128, 512], F32, tag="pso")
                            for fh in range(16):
                                nc.tensor.matmul(
                                    pso[:],
                                    oT[fh][:, st * 128:(st + 1) * 128],
                                    wo[fh][:],
                                    start=(fh == 0), stop=(fh == 15),
                                )
                            ou = osbp.tile([128, 512], F32, name="ou", tag="ou")
                            nc.scalar.copy(out=ou[:], in_=pso[:])
                            nc.sync.dma_start(
                                out=out[st * 128:(st + 1) * 128,
                                        eseg * 512:(eseg + 1) * 512],
                                in_=ou[:],
                            )
    return nc


# ---------------- host-side sharding ----------------
def _host_prep(input, freqs_cos, freqs_sin, w_in, w_out):
    x = np.asarray(input, np.float32).reshape(S, E)
    w_in = np.array(w_in, np.float32, copy=True)
    w_in[:, :E] *= np.float32(1.0 / np.sqrt(D))
    w_out = np.ascontiguousarray(np.asarray(w_out, np.float32))
    fc = np.asarray(freqs_cos, np.float32)
    fs = np.asarray(freqs_sin, np.float32)

    xpad = np.concatenate([np.zeros((W, E), np.float32), x], axis=0)
    ii = np.arange(128)[:, None]
    jj = np.arange(128)[None, :]
    maskLR = np.zeros((128, 256), np.float32)
    maskLR[:, :128] = np.where(jj >= ii, 0.0, NEG)
    maskLR[:, 128:] = np.where(jj <= ii, 0.0, NEG)

    in_maps = []
    for c in range(NCORES):
        sl = xpad[c * S_SH: c * S_SH + L]
        xTc = np.ascontiguousarray(sl.T)
        pos = np.arange(c * S_SH - W, c * S_SH + S_SH)
        posc = np.clip(pos, 0, S - 1)
        p = np.arange(c * S_SH, (c + 1) * S_SH)
        sfx = (-np.maximum(0, W - p)).astype(np.float32)[:, None]
        in_maps.append({
            "xT": xTc,
            "w_in": w_in,
            "w_out": w_out,
            "cos_k": np.ascontiguousarray(fc[posc]),
            "sin_k": np.ascontiguousarray(fs[posc]),
            "maskLR": maskLR,
            "sumfix": sfx,
        })
    return in_maps


def _get_program():
    if "nc" not in _CACHE:
        _CACHE["nc"] = _build_program()
    return _CACHE["nc"]


def kernel(**inputs) -> np.ndarray:
    from concourse import bass2jax

    nc = _get_program()
    in_maps = _host_prep(**inputs)
    results = bass2jax.run_bass_via_pjrt(nc, in_maps, n_cores=NCORES)
    shards = [results[c]["out"] for c in range(NCORES)]
    full = np.concatenate(shards, axis=0).reshape(B, S, E)
    return full.astype(np.float32)
